# revision 3
# baseline (speedup 1.0000x reference)
"""Trainium2 Bass kernel for a 3x3 stride-1 pad-1 conv2d (LoopConv2d).

Problem: x[16, 64, 112, 112] (f32), w[128, 64, 3, 3], b[128]
         -> out[16, 128, 112, 112]  (out = conv2d(x, w) + b)

Strategy
--------
Data-parallel over batch: 16 images / 8 cores = 2 images per core.

Per core, direct convolution as PE matmuls accumulating in PSUM:
  out[co, pix] += sum_ci w[co, ci, dy, dx] * x[ci, pix + (dy, dx)]

The contraction dim (C_IN = 64) only half-fills the 128-row PE array, so
input rows are parity-packed in SBUF: partition k < 64 holds channel k of
EVEN image rows, partition 64+k holds channel k of ODD rows, with row pair
(2s, 2s+1) sharing column-slot s.  A K=128 matmul over slot s then
contracts TWO vertical taps (dy, dy+1) at once.  Per output-row parity
group, 2 of the 3 vertical taps merge into one K=128 matmul; the third is
a K=64 matmul on one partition half (row-tiled, so the even-group and
odd-group leftovers can overlap on the PE).

Rows are padded to WP=114 (1 zero col each side) and the row-index space
is shifted by +2 (idx = row + 2, idx 0..115 -> 58 slots) so the dy=-1 tap
of output row 0 and dy=+1 of row 111 read zero slots - no edge branches.

Matmul free dim spans G=4 output rows of one parity = 456 columns <= 512
(one PSUM bank); the 2 junk columns per row (conv windows straddling row
ends) are simply not copied out.

dtype: float32r (fp32 bits, reduced-precision PE path, 1 cycle/row at
N>=256 vs 4 cycles/row for plain fp32).  Measured rel l2 err vs the f32
jax reference: ~1.4e-4.

Weights / bias / x are pre-arranged on host (numpy) into the exact SBUF
layouts so every DMA is a large contiguous copy.
"""

import numpy as np
import jax

import concourse.bass as bass
import concourse.tile as tile
from concourse import bacc, mybir
from concourse import bass2jax
from jax.experimental.shard_map import shard_map
from jax.sharding import Mesh, PartitionSpec

B, C_IN, H, W = 16, 64, 112, 112
C_OUT, KH, KW = 128, 3, 3
N_CORES = 8
WP = W + 2  # padded row width

F32 = mybir.dt.float32


def _prep_x(x: np.ndarray, h: int) -> np.ndarray:
    """[b, C_IN, h, W] f32 -> [b, 128, HP*WP + 2] parity-packed padded rows.

    idx = row + 2; slot s holds idx rows (2s, 2s+1) on partition halves
    (lower, upper); idx rows 0,1 (= real -2,-1) and h+2..h+3 are zeros.
    """
    nb = x.shape[0]
    hp = (h + 4) // 2  # slots
    xpad = np.zeros((nb, C_IN, 2 * hp, WP), dtype=np.float32)
    xpad[:, :, 2 : h + 2, 1 : W + 1] = x
    out = np.zeros((nb, 2 * C_IN, hp * WP + 2), dtype=np.float32)
    out[:, :C_IN, : hp * WP] = xpad[:, :, 0::2, :].reshape(nb, C_IN, hp * WP)
    out[:, C_IN:, : hp * WP] = xpad[:, :, 1::2, :].reshape(nb, C_IN, hp * WP)
    return out


def _prep_w(w: np.ndarray) -> np.ndarray:
    """[C_OUT, C_IN, 3, 3] -> [128, 9, C_OUT] stationary-operand configs.

    cfg 0..2  (even-parity K=128, dx=cfg):   lower w[:,:,1,dx], upper w[:,:,2,dx]
    cfg 3..5  (odd-parity  K=128, dx=cfg-3): lower w[:,:,0,dx], upper w[:,:,1,dx]
    cfg 6..8  (K=64 leftovers, dx=cfg-6):    lower w[:,:,2,dx] (odd dy=+1),
                                             upper w[:,:,0,dx] (even dy=-1)
    """
    wt = w.transpose(1, 2, 3, 0).astype(np.float32)  # [ci, kh, kw, co]
    out = np.empty((2 * C_IN, 9, C_OUT), dtype=np.float32)
    for dx in range(3):
        out[:C_IN, 0 + dx] = wt[:, 1, dx]
        out[C_IN:, 0 + dx] = wt[:, 2, dx]
        out[:C_IN, 3 + dx] = wt[:, 0, dx]
        out[C_IN:, 3 + dx] = wt[:, 1, dx]
        out[:C_IN, 6 + dx] = wt[:, 2, dx]
        out[C_IN:, 6 + dx] = wt[:, 0, dx]
    return out


def build(b_sh: int, h: int, mm_dt=mybir.dt.float32r, repeat: int = 1):
    """Build the per-core Bass program. h must be divisible by 8.

    repeat > 1 re-runs the whole conv back to back (for timing via
    wall-clock differences; output is just overwritten).
    """
    assert h % 8 == 0
    nt = h // 8  # pair-units (8 output rows each)
    hp = (h + 4) // 2  # slots
    xcols = hp * WP + 2
    nfree = 4 * WP  # matmul free dim (456)

    nc = bacc.Bacc(
        "TRN2", target_bir_lowering=False, debug=False, num_devices=N_CORES
    )
    x_d = nc.dram_tensor("xprep", [b_sh, 128, xcols], mm_dt, kind="ExternalInput").ap()
    w_d = nc.dram_tensor("wprep", [128, 9, C_OUT], mm_dt, kind="ExternalInput").ap()
    b_d = nc.dram_tensor("bias", [C_OUT, 1], F32, kind="ExternalInput").ap()
    o_d = nc.dram_tensor("out", [b_sh, C_OUT, h, W], F32, kind="ExternalOutput").ap()

    with tile.TileContext(nc) as tc:
        with (
            tc.tile_pool(name="wpool", bufs=1) as wpool,
            tc.tile_pool(name="xpool", bufs=2) as xpool,
            tc.tile_pool(name="stage", bufs=4) as stage,
            tc.tile_pool(name="psum", bufs=8, space="PSUM") as pspool,
        ):
            wt = wpool.tile([128, 9, C_OUT], mm_dt)
            bt = wpool.tile([C_OUT, 1], F32)
            nc.sync.dma_start(wt[:], w_d[:])
            nc.sync.dma_start(bt[:], b_d[:])

            for _rep in range(repeat):
                for b in range(b_sh):
                    xt = xpool.tile([128, xcols], mm_dt, tag="x")
                    nc.sync.dma_start(xt[:], x_d[b])

                    for t in range(nt):
                        # even group: out rows 8t, 8t+2, 8t+4, 8t+6
                        # odd  group: out rows 8t+1, ..., 8t+7
                        ps_e = pspool.tile([C_OUT, nfree], F32, tag="ps")
                        ps_o = pspool.tile([C_OUT, nfree], F32, tag="ps")
                        c_k128 = (4 * t + 1) * WP  # both parities
                        c_el = (4 * t) * WP  # even leftover (dy=-1, upper)
                        c_ol = (4 * t + 2) * WP  # odd leftover (dy=+1, lower)

                        for dx in range(3):
                            nc.tensor.matmul(
                                ps_e[:, :],
                                wt[:, dx, :],
                                xt[:, c_k128 + dx : c_k128 + dx + nfree],
                                start=(dx == 0),
                                stop=False,
                            )
                        for dx in range(3):
                            nc.tensor.matmul(
                                ps_o[:, :],
                                wt[:, 3 + dx, :],
                                xt[:, c_k128 + dx : c_k128 + dx + nfree],
                                start=(dx == 0),
                                stop=False,
                            )
                        # K=64 leftovers; adjacent row-disjoint pairs overlap
                        for dx in range(3):
                            nc.tensor.matmul(
                                ps_e[:, :],
                                wt[64:128, 6 + dx, :],
                                xt[64:128, c_el + dx : c_el + dx + nfree],
                                start=False,
                                stop=(dx == 2),
                            )
                            nc.tensor.matmul(
                                ps_o[:, :],
                                wt[0:64, 6 + dx, :],
                                xt[0:64, c_ol + dx : c_ol + dx + nfree],
                                start=False,
                                stop=(dx == 2),
                            )

                        # bias + evacuate PSUM -> staging [co, 4, 2, W]
                        st = stage.tile([C_OUT, 4, 2, W], F32, tag="st")
                        ps_e_v = ps_e[:].rearrange("p (g w) -> p g w", w=WP)[
                            :, :, 0:W
                        ]
                        ps_o_v = ps_o[:].rearrange("p (g w) -> p g w", w=WP)[
                            :, :, 0:W
                        ]
                        nc.scalar.activation(
                            st[:, :, 0, :],
                            ps_e_v,
                            mybir.ActivationFunctionType.Identity,
                            bias=bt[:, 0:1],
                        )
                        nc.vector.tensor_scalar_add(
                            st[:, :, 1, :], ps_o_v, bt[:, 0:1]
                        )
                        nc.sync.dma_start(o_d[b, :, 8 * t : 8 * t + 8, :], st[:])

    nc.compile()
    return nc


class Runner:
    """Persistent jitted shard_map executor for a compiled Bass program.

    Mirrors concourse.bass2jax.run_bass_via_pjrt's multi-core path but
    caches the jitted function so repeated calls skip re-tracing.
    """

    def __init__(self, nc, n_cores: int = N_CORES):
        bass2jax.install_neuronx_cc_hook()
        assert nc.dbg_addr is None
        self.nc = nc
        self.n_cores = n_cores
        partition_name = (
            nc.partition_id_tensor.name if nc.partition_id_tensor else None
        )
        in_names: list[str] = []
        out_names: list[str] = []
        out_avals: list[jax.core.ShapedArray] = []
        for alloc in nc.m.functions[0].allocations:
            if not isinstance(alloc, mybir.MemoryLocationSet):
                continue
            name = alloc.memorylocations[0].name
            if alloc.kind == "ExternalInput":
                if name != partition_name:
                    in_names.append(name)
            elif alloc.kind == "ExternalOutput":
                out_names.append(name)
                out_avals.append(
                    jax.core.ShapedArray(
                        tuple(alloc.tensor_shape), mybir.dt.np(alloc.dtype)
                    )
                )
        self.in_names = in_names
        self.out_names = out_names
        self.out_avals = out_avals
        n_params = len(in_names)
        n_outs = len(out_names)
        all_names = list(in_names) + list(out_names)
        if partition_name is not None:
            all_names.append(partition_name)
        all_names = tuple(all_names)

        def _body(*args):
            operands = list(args)
            if partition_name is not None:
                operands.append(bass2jax.partition_id_tensor())
            outs = bass2jax._bass_exec_p.bind(
                *operands,
                out_avals=tuple(out_avals),
                in_names=all_names,
                out_names=tuple(out_names),
                lowering_input_output_aliases=(),
                sim_require_finite=True,
                sim_require_nnan=True,
                nc=nc,
            )
            return tuple(outs)

        devices = jax.devices()[:n_cores]
        assert len(devices) == n_cores
        self.mesh = Mesh(np.asarray(devices), ("core",))
        in_specs = (PartitionSpec("core"),) * (n_params + n_outs)
        out_specs = (PartitionSpec("core"),) * n_outs
        donate = tuple(range(n_params, n_params + n_outs))
        self.fn = jax.jit(
            shard_map(
                _body,
                mesh=self.mesh,
                in_specs=in_specs,
                out_specs=out_specs,
                check_rep=False,
            ),
            donate_argnums=donate,
            keep_unused=True,
        )

    def concat_inputs(self, in_maps):
        return [
            np.concatenate([np.asarray(m[name]) for m in in_maps], axis=0)
            for name in self.in_names
        ]

    def zero_outs(self):
        return [
            np.zeros((self.n_cores * a.shape[0], *a.shape[1:]), a.dtype)
            for a in self.out_avals
        ]

    def call_raw(self, concat_in, zeros):
        """concat_in/zeros may be np or device arrays. Returns jax arrays."""
        return self.fn(*concat_in, *zeros)

    def __call__(self, in_maps):
        outs = self.call_raw(self.concat_inputs(in_maps), self.zero_outs())
        outs = [np.asarray(o) for o in outs]
        return [
            {
                name: outs[i].reshape(self.n_cores, *self.out_avals[i].shape)[c]
                for i, name in enumerate(self.out_names)
            }
            for c in range(self.n_cores)
        ]


_CACHE: dict = {}


def get_runner(repeat: int = 1) -> Runner:
    key = ("full", repeat)
    if key not in _CACHE:
        nc = build(B // N_CORES, H, repeat=repeat)
        _CACHE[key] = Runner(nc)
    return _CACHE[key]


def make_in_maps(x, w, b):
    b_sh = B // N_CORES
    wp = _prep_w(np.asarray(w))
    bp = np.asarray(b).astype(np.float32).reshape(C_OUT, 1)
    xp = _prep_x(np.asarray(x, dtype=np.float32), H)
    return [
        {"xprep": xp[i * b_sh : (i + 1) * b_sh], "wprep": wp, "bias": bp}
        for i in range(N_CORES)
    ]


def kernel(x, w, b):
    runner = get_runner()
    res = runner(make_in_maps(x, w, b))
    return np.concatenate([r["out"] for r in res], axis=0)


# revision 6
# speedup vs baseline: 81.3808x; 81.3808x over previous
"""Trainium2 Bass kernel for a 3x3 stride-1 pad-1 conv2d (LoopConv2d).

Problem: x[16, 64, 112, 112] (f32), w[128, 64, 3, 3], b[128]
         -> out[16, 128, 112, 112]  (out = conv2d(x, w) + b)

Strategy
--------
Data-parallel over batch: 16 images / 8 cores = 2 images per core.

Per core, direct convolution as PE matmuls accumulating in PSUM:
  out[co, pix] += sum_ci w[co, ci, dy, dx] * x[ci, pix + (dy, dx)]

The contraction dim (C_IN = 64) only half-fills the 128-row PE array, so
input rows are parity-packed in SBUF: partition k < 64 holds channel k of
EVEN image rows, partition 64+k holds channel k of ODD rows, with row pair
(2s, 2s+1) sharing column-slot s.  A K=128 matmul over slot s then
contracts TWO vertical taps (dy, dy+1) at once.  Per output-row parity
group, 2 of the 3 vertical taps merge into one K=128 matmul; the third is
a K=64 matmul on one partition half (row-tiled, so the even-group and
odd-group leftovers can overlap on the PE).

Rows are padded to WP=114 (1 zero col each side) and the row-index space
is shifted by +2 (idx = row + 2, idx 0..115 -> 58 slots) so the dy=-1 tap
of output row 0 and dy=+1 of row 111 read zero slots - no edge branches.

Matmul free dim spans G=4 output rows of one parity = 456 columns <= 512
(one PSUM bank); the 2 junk columns per row (conv windows straddling row
ends) are simply not copied out.

dtype: float32r (fp32 bits, reduced-precision PE path, 1 cycle/row at
N>=256 vs 4 cycles/row for plain fp32).  Measured rel l2 err vs the f32
jax reference: ~1.4e-4.

Weights / bias / x are pre-arranged on host (numpy) into the exact SBUF
layouts so every DMA is a large contiguous copy.
"""

import numpy as np
import jax

import concourse.bass as bass
import concourse.tile as tile
from concourse import bacc, mybir
from concourse import bass2jax
from jax.experimental.shard_map import shard_map
from jax.sharding import Mesh, PartitionSpec

B, C_IN, H, W = 16, 64, 112, 112
C_OUT, KH, KW = 128, 3, 3
N_CORES = 8
WP = W + 2  # padded row width

F32 = mybir.dt.float32


def _prep_x(x: np.ndarray, h: int) -> np.ndarray:
    """[b, C_IN, h, W] f32 -> [b, 128, HP*WP + 2] parity-packed padded rows.

    idx = row + 2; slot s holds idx rows (2s, 2s+1) on partition halves
    (lower, upper); idx rows 0,1 (= real -2,-1) and h+2..h+3 are zeros.
    """
    nb = x.shape[0]
    hp = (h + 4) // 2  # slots
    xpad = np.zeros((nb, C_IN, 2 * hp, WP), dtype=np.float32)
    xpad[:, :, 2 : h + 2, 1 : W + 1] = x
    out = np.zeros((nb, 2 * C_IN, hp * WP + 2), dtype=np.float32)
    out[:, :C_IN, : hp * WP] = xpad[:, :, 0::2, :].reshape(nb, C_IN, hp * WP)
    out[:, C_IN:, : hp * WP] = xpad[:, :, 1::2, :].reshape(nb, C_IN, hp * WP)
    return out


def _prep_w(w: np.ndarray) -> np.ndarray:
    """[C_OUT, C_IN, 3, 3] -> [128, 9, C_OUT] stationary-operand configs.

    cfg 0..2  (even-parity K=128, dx=cfg):   lower w[:,:,1,dx], upper w[:,:,2,dx]
    cfg 3..5  (odd-parity  K=128, dx=cfg-3): lower w[:,:,0,dx], upper w[:,:,1,dx]
    cfg 6..8  (K=64 leftovers, dx=cfg-6):    lower w[:,:,2,dx] (odd dy=+1),
                                             upper w[:,:,0,dx] (even dy=-1)
    """
    wt = w.transpose(1, 2, 3, 0).astype(np.float32)  # [ci, kh, kw, co]
    out = np.empty((2 * C_IN, 9, C_OUT), dtype=np.float32)
    for dx in range(3):
        out[:C_IN, 0 + dx] = wt[:, 1, dx]
        out[C_IN:, 0 + dx] = wt[:, 2, dx]
        out[:C_IN, 3 + dx] = wt[:, 0, dx]
        out[C_IN:, 3 + dx] = wt[:, 1, dx]
        out[:C_IN, 6 + dx] = wt[:, 2, dx]
        out[C_IN:, 6 + dx] = wt[:, 0, dx]
    return out


def build(b_sh: int, h: int, mm_dt=mybir.dt.float32r, repeat: int = 1, loop: int = 0):
    """Build the per-core Bass program. h must be divisible by 8.

    repeat > 1 re-runs the whole conv back to back (python-unrolled).
    loop > 0 wraps the conv in a hardware For_i loop running it `loop`
    times (for timing; output is just overwritten each iteration).
    """
    assert h % 8 == 0
    nt = h // 8  # pair-units (8 output rows each)
    hp = (h + 4) // 2  # slots
    xcols = hp * WP + 2
    nfree = 4 * WP  # matmul free dim (456)

    nc = bacc.Bacc(
        "TRN2", target_bir_lowering=False, debug=False, num_devices=N_CORES
    )
    x_d = nc.dram_tensor("xprep", [b_sh, 128, xcols], mm_dt, kind="ExternalInput").ap()
    w_d = nc.dram_tensor("wprep", [128, 9, C_OUT], mm_dt, kind="ExternalInput").ap()
    b_d = nc.dram_tensor("bias", [C_OUT, 1], F32, kind="ExternalInput").ap()
    o_d = nc.dram_tensor("out", [b_sh, C_OUT, h, W], F32, kind="ExternalOutput").ap()

    from contextlib import ExitStack, nullcontext

    with tile.TileContext(nc) as tc:
        with (
            tc.tile_pool(name="wpool", bufs=1) as wpool,
            tc.tile_pool(name="xpool", bufs=2) as xpool,
            tc.tile_pool(name="stage", bufs=4) as stage,
            tc.tile_pool(name="psum", bufs=8, space="PSUM") as pspool,
        ):
            wt = wpool.tile([128, 9, C_OUT], mm_dt)
            bt = wpool.tile([C_OUT, 1], F32)
            nc.sync.dma_start(wt[:], w_d[:])
            nc.sync.dma_start(bt[:], b_d[:])

            def emit_conv():
                for b in range(b_sh):
                    xt = xpool.tile([128, xcols], mm_dt, tag="x")
                    nc.sync.dma_start(xt[:], x_d[b])

                    for t in range(nt):
                        # even group: out rows 8t, 8t+2, 8t+4, 8t+6
                        # odd  group: out rows 8t+1, ..., 8t+7
                        ps_e = pspool.tile([C_OUT, nfree], F32, tag="ps")
                        ps_o = pspool.tile([C_OUT, nfree], F32, tag="ps")
                        c_k128 = (4 * t + 1) * WP  # both parities
                        c_el = (4 * t) * WP  # even leftover (dy=-1, upper)
                        c_ol = (4 * t + 2) * WP  # odd leftover (dy=+1, lower)

                        for dx in range(3):
                            nc.tensor.matmul(
                                ps_e[:, :],
                                wt[:, dx, :],
                                xt[:, c_k128 + dx : c_k128 + dx + nfree],
                                start=(dx == 0),
                                stop=False,
                            )
                        for dx in range(3):
                            nc.tensor.matmul(
                                ps_o[:, :],
                                wt[:, 3 + dx, :],
                                xt[:, c_k128 + dx : c_k128 + dx + nfree],
                                start=(dx == 0),
                                stop=False,
                            )
                        # K=64 leftovers; adjacent row-disjoint pairs overlap
                        for dx in range(3):
                            nc.tensor.matmul(
                                ps_e[:, :],
                                wt[64:128, 6 + dx, :],
                                xt[64:128, c_el + dx : c_el + dx + nfree],
                                start=False,
                                stop=(dx == 2),
                            )
                            nc.tensor.matmul(
                                ps_o[:, :],
                                wt[0:64, 6 + dx, :],
                                xt[0:64, c_ol + dx : c_ol + dx + nfree],
                                start=False,
                                stop=(dx == 2),
                            )

                        # bias + evacuate PSUM -> staging [co, 4, 2, W]
                        st = stage.tile([C_OUT, 4, 2, W], F32, tag="st")
                        ps_e_v = ps_e[:].rearrange("p (g w) -> p g w", w=WP)[
                            :, :, 0:W
                        ]
                        ps_o_v = ps_o[:].rearrange("p (g w) -> p g w", w=WP)[
                            :, :, 0:W
                        ]
                        nc.scalar.activation(
                            st[:, :, 0, :],
                            ps_e_v,
                            mybir.ActivationFunctionType.Identity,
                            bias=bt[:, 0:1],
                        )
                        nc.vector.tensor_scalar_add(
                            st[:, :, 1, :], ps_o_v, bt[:, 0:1]
                        )
                        nc.sync.dma_start(o_d[b, :, 8 * t : 8 * t + 8, :], st[:])

            if loop > 0:
                with tc.For_i(0, loop, 1, hint_engines=(mybir.EngineType.PE,)):
                    emit_conv()
            else:
                for _rep in range(repeat):
                    emit_conv()

    nc.compile()
    return nc


class Runner:
    """Persistent jitted shard_map executor for a compiled Bass program.

    Mirrors concourse.bass2jax.run_bass_via_pjrt's multi-core path but
    caches the jitted function so repeated calls skip re-tracing.
    """

    def __init__(self, nc, n_cores: int = N_CORES):
        bass2jax.install_neuronx_cc_hook()
        assert nc.dbg_addr is None
        self.nc = nc
        self.n_cores = n_cores
        partition_name = (
            nc.partition_id_tensor.name if nc.partition_id_tensor else None
        )
        in_names: list[str] = []
        out_names: list[str] = []
        out_avals: list[jax.core.ShapedArray] = []
        for alloc in nc.m.functions[0].allocations:
            if not isinstance(alloc, mybir.MemoryLocationSet):
                continue
            name = alloc.memorylocations[0].name
            if alloc.kind == "ExternalInput":
                if name != partition_name:
                    in_names.append(name)
            elif alloc.kind == "ExternalOutput":
                out_names.append(name)
                out_avals.append(
                    jax.core.ShapedArray(
                        tuple(alloc.tensor_shape), mybir.dt.np(alloc.dtype)
                    )
                )
        self.in_names = in_names
        self.out_names = out_names
        self.out_avals = out_avals
        n_params = len(in_names)
        n_outs = len(out_names)
        all_names = list(in_names) + list(out_names)
        if partition_name is not None:
            all_names.append(partition_name)
        all_names = tuple(all_names)

        def _body(*args):
            operands = list(args)
            if partition_name is not None:
                operands.append(bass2jax.partition_id_tensor())
            outs = bass2jax._bass_exec_p.bind(
                *operands,
                out_avals=tuple(out_avals),
                in_names=all_names,
                out_names=tuple(out_names),
                lowering_input_output_aliases=(),
                sim_require_finite=True,
                sim_require_nnan=True,
                nc=nc,
            )
            return tuple(outs)

        devices = jax.devices()[:n_cores]
        assert len(devices) == n_cores
        self.mesh = Mesh(np.asarray(devices), ("core",))
        in_specs = (PartitionSpec("core"),) * (n_params + n_outs)
        out_specs = (PartitionSpec("core"),) * n_outs
        donate = tuple(range(n_params, n_params + n_outs))
        self.fn = jax.jit(
            shard_map(
                _body,
                mesh=self.mesh,
                in_specs=in_specs,
                out_specs=out_specs,
                check_rep=False,
            ),
            donate_argnums=donate,
            keep_unused=True,
        )

    def concat_inputs(self, in_maps):
        return [
            np.concatenate([np.asarray(m[name]) for m in in_maps], axis=0)
            for name in self.in_names
        ]

    def zero_outs(self):
        return [
            np.zeros((self.n_cores * a.shape[0], *a.shape[1:]), a.dtype)
            for a in self.out_avals
        ]

    def call_raw(self, concat_in, zeros):
        """concat_in/zeros may be np or device arrays. Returns jax arrays."""
        return self.fn(*concat_in, *zeros)

    def __call__(self, in_maps):
        outs = self.call_raw(self.concat_inputs(in_maps), self.zero_outs())
        outs = [np.asarray(o) for o in outs]
        return [
            {
                name: outs[i].reshape(self.n_cores, *self.out_avals[i].shape)[c]
                for i, name in enumerate(self.out_names)
            }
            for c in range(self.n_cores)
        ]


_CACHE: dict = {}


def get_runner(repeat: int = 1, loop: int = 0) -> Runner:
    key = ("full", repeat, loop)
    if key not in _CACHE:
        nc = build(B // N_CORES, H, repeat=repeat, loop=loop)
        _CACHE[key] = Runner(nc)
    return _CACHE[key]


def make_in_maps(x, w, b):
    b_sh = B // N_CORES
    wp = _prep_w(np.asarray(w))
    bp = np.asarray(b).astype(np.float32).reshape(C_OUT, 1)
    xp = _prep_x(np.asarray(x, dtype=np.float32), H)
    return [
        {"xprep": xp[i * b_sh : (i + 1) * b_sh], "wprep": wp, "bias": bp}
        for i in range(N_CORES)
    ]


def kernel(x, w, b):
    runner = get_runner()
    res = runner(make_in_maps(x, w, b))
    return np.concatenate([r["out"] for r in res], axis=0)
